# revision 32
# baseline (speedup 1.0000x reference)
"""Trainium2 Bass kernel for nn_AttnDBGNNLayer (8-core SPMD).

kernel(**inputs) takes the FULL inputs (as produced by setup_inputs) and
returns the FULL output (new_A, new_B), distributing across 8 NeuronCores.

Design:
- q-rows of both attentions sharded 8-way (1024 rows/core); K/V computed
  replicated from a feature-major x0^T, with the K/V matmuls interleaved
  into the first attention q-group's loop (4-pr deadline lead) so the
  Scalar-engine EXP stream — the kernel's long pole — starts immediately;
  A and B q-groups interleaved so the TensorEngine always has independent
  work; single-pass unnormalized softmax (scores are tiny; no max
  subtraction; the K bias is dropped entirely — it adds a per-query
  constant to every score in a row, which row-softmax cancels exactly);
  out-projection folded into V (Wvo = Wout @ Wv); softmax row-sum
  accumulated on DVE+GpSimd (per-qg buffers, first-iteration copy instead
  of memset+add), finished with a ones-matmul; normalize batched
  (row-sums, then transpose+scale+store, then re-transposes) via
  PE-transpose + per-partition scale.
- normalized h rows (both types side by side, [R, 256]) are AllGathered in
  two row-halves so the first collective hides under the second attention
  q-group and the second under the half-0 aggregation; the gathered h
  lands in one SBUF tile (reusing the x0 buffer) swizzled so each 128-row
  source block is a matmul lhsT (8 parallel DMA chunks per half).
- message aggregation as dense count-matrix matmuls on RAW h:
  agg_g^T += h_blk^T @ C_g with C_g the per-core [8192 src, 1024 dst]
  edge-count matrix in fp8 (counts are small ints -> exact); lin_l (wl) is
  applied to the aggregate afterwards, so the three graphs share one
  gathered-h tile. C rows are host-permuted to match the AllGather row
  order and host-swizzled for contiguous streaming; the src-half-0 C of
  AB/BA is prefetched during the attention phase; aggregation runs in two
  src-half pieces (piece 0 needs only the first gather) with bf16 SBUF
  partials bridging the PSUM groups.
- degree/bias corrections (c0 + sum_g c1_g*deg_g) are rank-1/constant and
  are added on the host; lin_r accumulates into the same PSUM group as
  the wl matmuls; outputs staged bf16, transposed and corrected on host.
"""
import sys

if "/opt/trn_rl_repo" not in sys.path:
    sys.path.insert(0, "/opt/trn_rl_repo")

import numpy as np
import ml_dtypes

import concourse.bacc as bacc
import concourse.tile as tile
import concourse.mybir as mybir
from concourse import bass_utils

BF16 = ml_dtypes.bfloat16
FP8 = ml_dtypes.float8_e4m3

N = 8192
D = 128
NCORES = 8
R = N // NCORES       # 1024 rows per core
QG = 512              # q-group width
KB = N // 128         # 64 k-blocks
SCALE = 1.0 / np.sqrt(np.float32(D))

F32 = mybir.dt.float32
BF = mybir.dt.bfloat16
F8 = mybir.dt.float8e4

G = ("AB", "BA", "AA")
GI = {g: i for i, g in enumerate(G)}
SRC_T = {"AB": "A", "BA": "B", "AA": "A"}
GRAPHS_OF = {"A": ("BA", "AA"), "B": ("AB",)}

# bf16 weight blob layout: [128,128] slices
WB_ORDER = ["wqT_A", "wkT_A", "wvoT_A", "wqT_B", "wkT_B", "wvoT_B",
            "wlT_AB", "wlT_BA", "wlT_AA", "wrT_A", "wrT_B"]
# f32 col blob: [128, 4]
CB_ORDER = ["bqs_A", "bk_A", "bqs_B", "bk_B"]

_PROG_CACHE = {}


def build_program(dbg=False, stage=3):
    nc = bacc.Bacc("TRN2", target_bir_lowering=False, debug=False,
                   num_devices=NCORES)

    x0t = {t: nc.dram_tensor(f"x0t_{t}", [128, N], BF, kind="ExternalInput")
           for t in "AB"}
    x0q = {t: nc.dram_tensor(f"x0q_{t}", [128, R], BF, kind="ExternalInput")
           for t in "AB"}
    wblob = nc.dram_tensor("wblob", [128, 128 * len(WB_ORDER)], BF,
                           kind="ExternalInput")
    cblob = nc.dram_tensor("cblob", [128, len(CB_ORDER)], F32,
                           kind="ExternalInput")
    ct = {g: nc.dram_tensor(f"ct_{g}", [1024, 8 * R], F8,
                            kind="ExternalInput") for g in G}
    out_d = {t: nc.dram_tensor(f"out_{t}", [128, R], BF,
                               kind="ExternalOutput") for t in "AB"}
    dbg_d = {}
    if dbg:
        for t in "AB":
            dbg_d[f"ht_{t}"] = nc.dram_tensor(f"dbg_ht_{t}", [128, R], BF,
                                              kind="ExternalOutput")

    h_loc = nc.dram_tensor("h_loc", [R, 256], BF)
    h_all = nc.dram_tensor("h_all", [N, 256], BF, addr_space="Shared")

    with tile.TileContext(nc) as tc:
        with (
            tc.tile_pool(name="const", bufs=1) as cp,
            tc.tile_pool(name="big", bufs=1) as bp,
            tc.tile_pool(name="pt", bufs=4) as ptp,
            tc.tile_pool(name="ctp", bufs=8) as ctp,
            tc.tile_pool(name="ps_s", bufs=2, space="PSUM") as ps_s,
            tc.tile_pool(name="ps_u", bufs=2, space="PSUM") as ps_u,
            tc.tile_pool(name="ps_sm", bufs=2, space="PSUM") as ps_sm,
        ):
            # ---------------- inputs: small blobs first, then chunked x0
            wb = cp.tile([128, 128 * len(WB_ORDER)], BF, tag="wb")
            nc.sync.dma_start(out=wb[:], in_=wblob[:])
            W = {k: wb[:, i * 128:(i + 1) * 128]
                 for i, k in enumerate(WB_ORDER)}
            cb = cp.tile([128, len(CB_ORDER)], F32, tag="cb")
            nc.sync.dma_start(out=cb[:], in_=cblob[:])
            C = {k: cb[:, i:i + 1] for i, k in enumerate(CB_ORDER)}
            x0q_s = {}
            for t in "AB":
                x0q_s[t] = bp.tile([128, R], BF, tag=f"x0q_{t}",
                                   name=f"x0q_{t}")
                nc.sync.dma_start(out=x0q_s[t][:], in_=x0q[t][:])

            # x0comb holds x0^T of A (cols 0..8191) and B (cols 8192..);
            # the same 4MB buffer is later reused for the gathered h (hsb).
            x0comb = bp.tile([128, 2 * N], BF, tag="big", name="x0comb")
            for ch in range(4):
                for t, toff in (("A", 0), ("B", N)):
                    nc.sync.dma_start(
                        out=x0comb[:, toff + ch * 2048:toff + (ch + 1) * 2048],
                        in_=x0t[t][:, ch * 2048:(ch + 1) * 2048])

            ident = cp.tile([128, 128], BF, tag="ident")
            from concourse.masks import make_identity
            make_identity(nc, ident[:])
            ones_col = cp.tile([128, 1], BF, tag="ones_col")
            nc.vector.memset(ones_col[:], 1.0)

            ht = {t: bp.tile([128, R], BF, tag=f"ht_{t}", name=f"ht_{t}")
                  for t in "AB"}

            # ---------------- QKV emit helpers; the bulk is interleaved
            # into the qg0 attention loop so the Scalar EXP stream (the
            # kernel's long pole) starts as early as possible.
            TOFF = {"A": 0, "B": N}
            kt = {}
            vt = {}
            qt = {}
            for t in "AB":
                kt[t] = bp.tile([128, N], BF, tag=f"kt_{t}", name=f"kt_{t}")
                vt[t] = bp.tile([128, N], BF, tag=f"vt_{t}", name=f"vt_{t}")
                qt[t] = bp.tile([128, R], BF, tag=f"qt_{t}", name=f"qt_{t}")

            def emit_k(t, j):
                ps = ps_sm.tile([128, 512], F32, tag="sm",
                                name=f"kps_{t}_{j}")
                nc.tensor.matmul(
                    ps[:], lhsT=W[f"wkT_{t}"],
                    rhs=x0comb[:, TOFF[t] + j * 512:TOFF[t] + (j + 1) * 512],
                    start=True, stop=True)
                nc.vector.tensor_copy(kt[t][:, j * 512:(j + 1) * 512], ps[:])

            def emit_v(t, vg):
                ps = ps_sm.tile([128, 512], F32, tag="sm",
                                name=f"vps_{t}_{vg}")
                for i in range(4):
                    nb = vg * 4 + i
                    nc.tensor.matmul(
                        ps[:, i * 128:(i + 1) * 128],
                        lhsT=x0comb[:, TOFF[t] + nb * 128:
                                     TOFF[t] + (nb + 1) * 128],
                        rhs=W[f"wvoT_{t}"], start=True, stop=True)
                nc.vector.tensor_copy(vt[t][:, vg * 512:(vg + 1) * 512],
                                      ps[:])

            def emit_q(t, j):
                ps = ps_sm.tile([128, 512], F32, tag="sm",
                                name=f"qps_{t}_{j}")
                nc.tensor.matmul(
                    ps[:], lhsT=W[f"wqT_{t}"],
                    rhs=x0q_s[t][:, j * 512:(j + 1) * 512],
                    start=True, stop=True)
                nc.scalar.activation(
                    qt[t][:, j * 512:(j + 1) * 512], ps[:],
                    mybir.ActivationFunctionType.Identity,
                    bias=C[f"bqs_{t}"], scale=float(SCALE))

            for t in "AB":
                emit_q(t, 0)
            # prefetch the src-half-0 count matrices of AB/BA during the
            # attention phase (AA0-3 + the half-1 tiles stream into freed
            # bufs). Triggered from the scalar queue after the Q copies so
            # the 8MB doesn't compete with x0's input DMA at kernel start.
            ct_sb = {}
            for g in ("AB", "BA"):
                for scg in range(4):
                    ct_t = ctp.tile([128, 8 * R], F8, tag="ct",
                                    name=f"ct_{g}_{scg}")
                    nc.sync.dma_start(
                        out=ct_t[:],
                        in_=ct[g][scg * 128:(scg + 1) * 128, :])
                    ct_sb[(g, scg)] = ct_t
            for j in (0, 1):
                for t in "AB":
                    emit_k(t, j)
            for vg in (0, 1):
                for t in "AB":
                    emit_v(t, vg)

            # ---------------- attention, A/B interleaved; h gathered per half
            hsb = None
            for qg in range(R // QG):
                q_sl = slice(qg * QG, (qg + 1) * QG)
                ut_ps = {}
                racc0 = {}
                racc1 = {}
                for t in "AB":
                    ut_ps[t] = ps_u.tile([128, QG], F32, tag="ut",
                                         name=f"utps_{t}_{qg}")
                    racc0[t] = bp.tile([128, 2 * QG], BF,
                                       tag=f"racc0_{t}_{qg}",
                                       name=f"racc0_{t}_{qg}")
                    racc1[t] = bp.tile([128, 2 * QG], BF,
                                       tag=f"racc1_{t}_{qg}",
                                       name=f"racc1_{t}_{qg}")
                for pr in range(KB // 2):
                    kb0 = 2 * pr
                    for t in "AB":
                        sc = ps_s.tile([128, 1024], F32, tag="sc",
                                       name=f"sc_{t}_{pr}")
                        nc.tensor.matmul(sc[:, :512],
                                         lhsT=kt[t][:, kb0 * 128:(kb0 + 1) * 128],
                                         rhs=qt[t][:, q_sl],
                                         start=True, stop=True)
                        nc.tensor.matmul(sc[:, 512:],
                                         lhsT=kt[t][:, (kb0 + 1) * 128:(kb0 + 2) * 128],
                                         rhs=qt[t][:, q_sl],
                                         start=True, stop=True)
                        pt = ptp.tile([128, 1024], BF, tag="pt",
                                      name=f"pt_{t}_{pr}")
                        nc.scalar.activation(pt[:], sc[:],
                                             mybir.ActivationFunctionType.Exp)
                        nc.tensor.matmul(ut_ps[t][:],
                                         lhsT=vt[t][:, kb0 * 128:(kb0 + 1) * 128],
                                         rhs=pt[:, :512],
                                         start=(pr == 0), stop=False)
                        nc.tensor.matmul(ut_ps[t][:],
                                         lhsT=vt[t][:, (kb0 + 1) * 128:(kb0 + 2) * 128],
                                         rhs=pt[:, 512:],
                                         start=False, stop=(pr == KB // 2 - 1))
                        if pr % 4 != 3:
                            if pr == 0:
                                nc.vector.tensor_copy(racc0[t][:], pt[:])
                            else:
                                nc.vector.tensor_add(racc0[t][:], racc0[t][:],
                                                     pt[:])
                        else:
                            if pr == 3:
                                nc.gpsimd.tensor_copy(racc1[t][:], pt[:])
                            else:
                                nc.gpsimd.tensor_tensor(racc1[t][:],
                                                        racc1[t][:],
                                                        pt[:],
                                                        op=mybir.AluOpType.add)
                    if qg == 0:
                        # dribble the remaining K/V tiles into the loop,
                        # staying ~4 prs ahead of their first consumer
                        idx = pr // 2 + 2
                        if idx <= 15:
                            et = "A" if pr % 2 == 0 else "B"
                            emit_k(et, idx)
                            emit_v(et, idx)
                        if pr == 5:
                            emit_q("A", 1)
                            emit_q("B", 1)

                # normalize + both orientations of h; row-major h -> h_loc.
                # Sub-major so a row range finishes for both types early,
                # letting the partial AllGathers launch mid-normalize.
                ut_sb = {}
                for t in "AB":
                    ut_sb[t] = bp.tile([128, QG], BF, tag=f"ut_sb_{t}",
                                       name=f"ut_sb_{t}_{qg}")
                    nc.vector.tensor_copy(ut_sb[t][:], ut_ps[t][:])
                if hsb is None:
                    hsb = bp.tile([128, 2 * N], BF, tag="big", name="hsb")

                def gather_piece(r0, r1, b0, b1):
                    # AllGather h_loc rows [r0, r1) of each core; land the
                    # result (permuted-space blocks [b0, b1)) into hsb,
                    # swizzled: partition = row-within-block, free = (blk, td)
                    nrow = r1 - r0
                    g0 = b0 * 128
                    nc.gpsimd.collective_compute(
                        "AllGather", mybir.AluOpType.bypass,
                        replica_groups=[list(range(NCORES))],
                        ins=[h_loc[r0:r1, :]],
                        outs=[h_all[g0:g0 + nrow * NCORES, :]])
                    for pc in range(b0 // 4, b1 // 4):
                        nc.sync.dma_start(
                            out=hsb[:, pc * 1024:(pc + 1) * 1024]
                            .rearrange("s (b d) -> s b d", d=256),
                            in_=h_all[pc * 512:(pc + 1) * 512, :]
                            .rearrange("(b s) d -> s b d", s=128))

                # batched: all row-sums, then transpose+scale+store (with
                # the partial gathers launched as their rows land), then the
                # feature-major re-transposes.
                rinv = {}
                for sub in range(QG // 128):
                    s_sl = slice(sub * 128, (sub + 1) * 128)
                    for t in "AB":
                        rp = ps_sm.tile([128, 512], F32, tag="sm", name="rp")
                        nc.tensor.matmul(rp[:, :1], lhsT=racc0[t][:, s_sl],
                                         rhs=ones_col[:], start=True,
                                         stop=False)
                        nc.tensor.matmul(rp[:, :1],
                                         lhsT=racc0[t][:, 512 + sub * 128:
                                                      512 + (sub + 1) * 128],
                                         rhs=ones_col[:], start=False,
                                         stop=False)
                        nc.tensor.matmul(rp[:, :1], lhsT=racc1[t][:, s_sl],
                                         rhs=ones_col[:], start=False,
                                         stop=False)
                        nc.tensor.matmul(rp[:, :1],
                                         lhsT=racc1[t][:, 512 + sub * 128:
                                                      512 + (sub + 1) * 128],
                                         rhs=ones_col[:], start=False,
                                         stop=True)
                        rv = bp.tile([128, 1], F32, tag="rinv", bufs=8,
                                     name=f"rinv_{t}_{sub}")
                        nc.vector.reciprocal(rv[:], rp[:, :1])
                        rinv[(t, sub)] = rv
                hns = {}
                for sub in range(QG // 128):
                    s_sl = slice(sub * 128, (sub + 1) * 128)
                    for t in "AB":
                        tyoff = 0 if t == "A" else 128
                        tp = ps_sm.tile([128, 512], BF, tag="sm", name="tp")
                        nc.tensor.transpose(tp[:, :128], ut_sb[t][:, s_sl],
                                            ident[:])
                        hn = bp.tile([128, 128], BF, tag=f"hn_{t}", bufs=4,
                                     name=f"hn_{t}_{sub}")
                        nc.vector.tensor_scalar_mul(hn[:], tp[:, :128],
                                                    rinv[(t, sub)][:, :])
                        nc.sync.dma_start(
                            out=h_loc[qg * QG + sub * 128:
                                      qg * QG + (sub + 1) * 128,
                                      tyoff:tyoff + 128],
                            in_=hn[:])
                        hns[(t, sub)] = hn

                for sub in range(QG // 128):
                    for t in "AB":
                        tp2 = ps_sm.tile([128, 512], BF, tag="sm", name="tp2")
                        nc.tensor.transpose(tp2[:, :128], hns[(t, sub)][:],
                                            ident[:])
                        nc.vector.tensor_copy(
                            ht[t][:, qg * QG + sub * 128:
                                  qg * QG + (sub + 1) * 128],
                            tp2[:, :128])
                if qg == 0:
                    gather_piece(0, 512, 0, 32)
                else:
                    gather_piece(512, 1024, 32, 64)

            if dbg:
                for t in "AB":
                    nc.sync.dma_start(out=dbg_d[f"ht_{t}"][:], in_=ht[t][:])

            # ---------------- phase 2: dense count-matrix aggregation
            # agg_g^T[d, dst] = sum_blk h_blk^T @ C_g_blk ; then
            # out_t^T = sum_g wl_g @ agg_g^T + wr_t @ h_t^T  (corr on host)
            # Split per src-half: the half-0 matmuls only need AllGather#1,
            # so they run while AllGather#2 is still in flight.
            AGGSB_TAG = {"AB": "racc0_A_0", "BA": "racc0_B_0",
                         "AA": "racc1_A_0"}
            PIECES = ((0, 4), (4, 8))
            partial = {}
            aggsb = {}
            for pi in range(2 if stage >= 2 else 0):
                lo, hi = PIECES[pi]
                for g in G:
                    tyoff = 0 if SRC_T[g] == "A" else 128
                    agg = ps_s.tile([128, 1024], F32, tag="sc",
                                    name=f"agg_{g}_{pi}")
                    for scg in range(lo, hi):
                        if (g, scg) in ct_sb:
                            ct_t = ct_sb[(g, scg)]
                        else:
                            ct_t = ctp.tile([128, 8 * R], F8, tag="ct",
                                            name=f"ct_{g}_{scg}")
                            nc.sync.dma_start(
                                out=ct_t[:],
                                in_=ct[g][scg * 128:(scg + 1) * 128, :])
                        for sb in range(8):
                            blk = scg * 8 + sb
                            lt = hsb[:, blk * 256 + tyoff:
                                     blk * 256 + tyoff + 128]
                            for h in range(2):
                                nc.tensor.matmul(
                                    agg[:, h * 512:(h + 1) * 512],
                                    lhsT=lt,
                                    rhs=ct_t[:, sb * R + h * 512:
                                             sb * R + (h + 1) * 512],
                                    start=(scg == lo and sb == 0),
                                    stop=(scg == hi - 1 and sb == 7))
                    if pi == 0:
                        p = ptp.tile([128, 1024], BF, tag="pt",
                                     name=f"part_{g}")
                        nc.vector.tensor_copy(p[:], agg[:])
                        partial[g] = p
                    else:
                        asb = bp.tile([128, 1024], BF, tag=AGGSB_TAG[g],
                                      name=f"aggsb_{g}")
                        nc.vector.tensor_add(asb[:], partial[g][:], agg[:])
                        aggsb[g] = asb
                        # emit a target's output chain as soon as its last
                        # graph aggregate is ready (B after AB, A after AA)
                        t = {"AB": "B", "AA": "A"}.get(g)
                        if t is None:
                            continue
                        po = []
                        for h in range(2):
                            po_t = ps_u.tile([128, 512], F32, tag="ut",
                                             name=f"po_{t}_{h}")
                            po.append(po_t)
                        for h in range(2):
                            nc.tensor.matmul(
                                po[h][:], lhsT=W[f"wrT_{t}"],
                                rhs=ht[t][:, h * 512:(h + 1) * 512],
                                start=True, stop=False)
                        ggs = GRAPHS_OF[t]
                        for gg in ggs:
                            for h in range(2):
                                nc.tensor.matmul(
                                    po[h][:], lhsT=W[f"wlT_{gg}"],
                                    rhs=aggsb[gg][:, h * 512:(h + 1) * 512],
                                    start=False, stop=(gg == ggs[-1]))
                        for h in range(2):
                            osb = bp.tile([128, 512], BF, tag="osb",
                                          name=f"osb_{t}_{h}")
                            nc.vector.tensor_copy(osb[:], po[h][:])
                            nc.sync.dma_start(
                                out=out_d[t][:, h * 512:(h + 1) * 512],
                                in_=osb[:])

    nc.compile()
    return nc


# ---------------------------------------------------------------- host prep

def _row_perm():
    """node id -> gathered-h row under the half AllGather layout."""
    n = np.arange(N)
    c = n >> 10
    w = n & 1023
    return (w >> 9) * 4096 + c * 512 + (w & 511)


def _prep(inputs, dbg=False):
    ins = {k: np.asarray(v) for k, v in inputs.items()}

    def bf(x):
        return np.ascontiguousarray(np.asarray(x, np.float32)).astype(BF16)

    com = {}
    for t in "AB":
        iw = ins[f"inW_{t}"].astype(np.float32)
        ib = ins[f"inB_{t}"].astype(np.float32)
        ow = ins[f"outW_{t}"].astype(np.float32)
        ob = ins[f"outB_{t}"].astype(np.float32)
        com[f"wqT_{t}"] = iw[0:128].T
        com[f"wkT_{t}"] = iw[128:256].T
        com[f"wvoT_{t}"] = (ow @ iw[256:384]).T
        com[f"bqs_{t}"] = ib[0:128] * SCALE
        com[f"bk_{t}"] = ib[128:256]
        com[f"bout_eff_{t}"] = ow @ ib[256:384] + ob
    for g in G:
        com[f"wlT_{g}"] = ins[f"wl_{g}"].astype(np.float32).T
        com[f"c1_{g}"] = (ins[f"wl_{g}"].astype(np.float32)
                          @ com[f"bout_eff_{SRC_T[g]}"])
    com["wrT_B"] = ins["wr_AB"].astype(np.float32).T
    com["wrT_A"] = (ins["wr_BA"] + ins["wr_AA"]).astype(np.float32).T
    com["c0_B"] = (ins["bl_AB"].astype(np.float32)
                   + ins["wr_AB"].astype(np.float32) @ com["bout_eff_B"])
    com["c0_A"] = (ins["bl_BA"].astype(np.float32)
                   + ins["bl_AA"].astype(np.float32)
                   + (ins["wr_BA"] + ins["wr_AA"]).astype(np.float32)
                   @ com["bout_eff_A"])

    wblob = bf(np.concatenate([com[k] for k in WB_ORDER], axis=1))
    cblob = np.stack([com[k] for k in CB_ORDER], axis=1).astype(np.float32)

    x0T = {t: np.ascontiguousarray(
        ins[f"x_{t}"][:, 0, :].astype(np.float32).T).astype(BF16)
        for t in "AB"}

    perm = _row_perm()
    cts = {}
    degs = {}
    for g in G:
        src = np.asarray(ins[f"ei_{g}"][0], np.int64)
        dst = np.asarray(ins[f"ei_{g}"][1], np.int64)
        per_core = []
        dgs = []
        for c in range(NCORES):
            sel = (dst >> 10) == c
            s_c = perm[src[sel]]          # permuted gathered-h rows
            d_c = dst[sel] - c * R
            cmat = np.zeros((N, R), np.float32)
            np.add.at(cmat, (s_c, d_c), 1.0)
            swz = np.ascontiguousarray(
                cmat.reshape(8, 8, 128, R).transpose(0, 2, 1, 3)
                .reshape(1024, 8 * R))
            per_core.append(swz.astype(FP8))
            dgs.append(np.bincount(d_c, minlength=R).astype(np.float32))
        cts[g] = per_core
        degs[g] = dgs

    # host-side degree/bias correction, [R, 128] per (target, core)
    corr = {}
    for t in "AB":
        for c in range(NCORES):
            acc = np.broadcast_to(com[f"c0_{t}"], (R, 128)).copy()
            for g in GRAPHS_OF[t]:
                acc += np.outer(degs[g][c], com[f"c1_{g}"])
            corr[(t, c)] = acc.astype(np.float32)

    in_maps = []
    for c in range(NCORES):
        m = {"wblob": wblob, "cblob": cblob}
        for t in "AB":
            m[f"x0t_{t}"] = x0T[t]
            m[f"x0q_{t}"] = np.ascontiguousarray(x0T[t][:, c * R:(c + 1) * R])
        for g in G:
            m[f"ct_{g}"] = cts[g][c]
        in_maps.append(m)
    return in_maps, corr


def kernel(**inputs):
    in_maps, corr = _prep(inputs)
    if "prog" not in _PROG_CACHE:
        _PROG_CACHE["prog"] = build_program()
    nc = _PROG_CACHE["prog"]
    res = bass_utils.run_bass_kernel_spmd(
        nc, in_maps, core_ids=list(range(NCORES)))
    x_A = np.asarray(inputs["x_A"], np.float32)
    x_B = np.asarray(inputs["x_B"], np.float32)
    new_A = x_A.copy()
    new_B = x_B.copy()
    for c in range(NCORES):
        new_A[c * R:(c + 1) * R, 0, :] = (res.results[c]["out_A"].T
                                          + corr[("A", c)])
        new_B[c * R:(c + 1) * R, 0, :] = (res.results[c]["out_B"].T
                                          + corr[("B", c)])
    return new_A, new_B


# revision 33
# speedup vs baseline: 1.0711x; 1.0711x over previous
"""Trainium2 Bass kernel for nn_AttnDBGNNLayer (8-core SPMD).

kernel(**inputs) takes the FULL inputs (as produced by setup_inputs) and
returns the FULL output (new_A, new_B), distributing across 8 NeuronCores.

Design:
- q-rows of both attentions sharded 8-way (1024 rows/core); K/V computed
  replicated from a feature-major x0^T, with the K/V matmuls interleaved
  into the first attention q-group's loop (4-pr deadline lead) so the
  Scalar-engine EXP stream — the kernel's long pole — starts immediately;
  A and B q-groups interleaved so the TensorEngine always has independent
  work; single-pass unnormalized softmax (scores are tiny; no max
  subtraction; the K bias is dropped entirely — it adds a per-query
  constant to every score in a row, which row-softmax cancels exactly);
  out-projection folded into V (Wvo = Wout @ Wv); softmax row-sum
  accumulated on DVE+GpSimd (per-qg buffers, first-iteration copy instead
  of memset+add), finished with a ones-matmul; normalize batched
  (row-sums, then transpose+scale+store, then re-transposes) via
  PE-transpose + per-partition scale.
- normalized h rows (both types side by side, [R, 256]) are AllGathered in
  two row-halves so the first collective hides under the second attention
  q-group and the second under the half-0 aggregation; the gathered h
  lands in one SBUF tile (reusing the x0 buffer) swizzled so each 128-row
  source block is a matmul lhsT (8 parallel DMA chunks per half).
- message aggregation as dense count-matrix matmuls on RAW h:
  agg_g^T += h_blk^T @ C_g with C_g the per-core [8192 src, 1024 dst]
  edge-count matrix in fp8 (counts are small ints -> exact); lin_l (wl) is
  applied to the aggregate afterwards, so the three graphs share one
  gathered-h tile. C rows are host-permuted to match the AllGather row
  order and host-swizzled for contiguous streaming; the src-half-0 C of
  AB/BA is prefetched during the attention phase; aggregation runs in two
  src-half pieces (piece 0 needs only the first gather) with bf16 SBUF
  partials bridging the PSUM groups.
- degree/bias corrections (c0 + sum_g c1_g*deg_g) are rank-1/constant and
  are added on the host; lin_r accumulates into the same PSUM group as
  the wl matmuls; outputs staged bf16, transposed and corrected on host.
"""
import sys

if "/opt/trn_rl_repo" not in sys.path:
    sys.path.insert(0, "/opt/trn_rl_repo")

import numpy as np
import ml_dtypes

import concourse.bacc as bacc
import concourse.tile as tile
import concourse.mybir as mybir
from concourse import bass_utils

BF16 = ml_dtypes.bfloat16
FP8 = ml_dtypes.float8_e4m3

N = 8192
D = 128
NCORES = 8
R = N // NCORES       # 1024 rows per core
QG = 512              # q-group width
KB = N // 128         # 64 k-blocks
SCALE = 1.0 / np.sqrt(np.float32(D))

F32 = mybir.dt.float32
BF = mybir.dt.bfloat16
F8 = mybir.dt.float8e4

G = ("AB", "BA", "AA")
GI = {g: i for i, g in enumerate(G)}
SRC_T = {"AB": "A", "BA": "B", "AA": "A"}
GRAPHS_OF = {"A": ("BA", "AA"), "B": ("AB",)}

# bf16 weight blob layout: [128,128] slices
WB_ORDER = ["wqT_A", "wkT_A", "wvoT_A", "wqT_B", "wkT_B", "wvoT_B",
            "wlT_AB", "wlT_BA", "wlT_AA", "wrT_A", "wrT_B"]
# f32 col blob: [128, 4]
CB_ORDER = ["bqs_A", "bk_A", "bqs_B", "bk_B"]

_PROG_CACHE = {}


def build_program(dbg=False, stage=3):
    nc = bacc.Bacc("TRN2", target_bir_lowering=False, debug=False,
                   num_devices=NCORES)

    x0t = {t: nc.dram_tensor(f"x0t_{t}", [128, N], BF, kind="ExternalInput")
           for t in "AB"}
    x0q = {t: nc.dram_tensor(f"x0q_{t}", [128, R], BF, kind="ExternalInput")
           for t in "AB"}
    wblob = nc.dram_tensor("wblob", [128, 128 * len(WB_ORDER)], BF,
                           kind="ExternalInput")
    cblob = nc.dram_tensor("cblob", [128, len(CB_ORDER)], F32,
                           kind="ExternalInput")
    ct = {g: nc.dram_tensor(f"ct_{g}", [1024, 8 * R], F8,
                            kind="ExternalInput") for g in G}
    out_d = {t: nc.dram_tensor(f"out_{t}", [128, R], BF,
                               kind="ExternalOutput") for t in "AB"}
    dbg_d = {}
    if dbg:
        for t in "AB":
            dbg_d[f"ht_{t}"] = nc.dram_tensor(f"dbg_ht_{t}", [128, R], BF,
                                              kind="ExternalOutput")

    h_loc = nc.dram_tensor("h_loc", [R, 256], BF)
    h_all = nc.dram_tensor("h_all", [N, 256], BF, addr_space="Shared")

    with tile.TileContext(nc) as tc:
        with (
            tc.tile_pool(name="const", bufs=1) as cp,
            tc.tile_pool(name="big", bufs=1) as bp,
            tc.tile_pool(name="pt", bufs=4) as ptp,
            tc.tile_pool(name="ctp", bufs=8) as ctp,
            tc.tile_pool(name="ps_s", bufs=2, space="PSUM") as ps_s,
            tc.tile_pool(name="ps_u", bufs=2, space="PSUM") as ps_u,
            tc.tile_pool(name="ps_sm", bufs=2, space="PSUM") as ps_sm,
        ):
            # ---------------- inputs: small blobs first, then chunked x0
            wb = cp.tile([128, 128 * len(WB_ORDER)], BF, tag="wb")
            nc.sync.dma_start(out=wb[:], in_=wblob[:])
            W = {k: wb[:, i * 128:(i + 1) * 128]
                 for i, k in enumerate(WB_ORDER)}
            cb = cp.tile([128, len(CB_ORDER)], F32, tag="cb")
            nc.sync.dma_start(out=cb[:], in_=cblob[:])
            C = {k: cb[:, i:i + 1] for i, k in enumerate(CB_ORDER)}
            x0q_s = {}
            for t in "AB":
                x0q_s[t] = bp.tile([128, R], BF, tag=f"x0q_{t}",
                                   name=f"x0q_{t}")
                nc.sync.dma_start(out=x0q_s[t][:], in_=x0q[t][:])

            # x0comb holds x0^T of A (cols 0..8191) and B (cols 8192..);
            # the same 4MB buffer is later reused for the gathered h (hsb).
            x0comb = bp.tile([128, 2 * N], BF, tag="big", name="x0comb")
            for ch in range(4):
                for t, toff in (("A", 0), ("B", N)):
                    nc.sync.dma_start(
                        out=x0comb[:, toff + ch * 2048:toff + (ch + 1) * 2048],
                        in_=x0t[t][:, ch * 2048:(ch + 1) * 2048])

            ident = cp.tile([128, 128], BF, tag="ident")
            from concourse.masks import make_identity
            make_identity(nc, ident[:])
            ones_col = cp.tile([128, 1], BF, tag="ones_col")
            nc.vector.memset(ones_col[:], 1.0)

            ht = {t: bp.tile([128, R], BF, tag=f"ht_{t}", name=f"ht_{t}")
                  for t in "AB"}

            # ---------------- QKV emit helpers; the bulk is interleaved
            # into the qg0 attention loop so the Scalar EXP stream (the
            # kernel's long pole) starts as early as possible.
            TOFF = {"A": 0, "B": N}
            kt = {}
            vt = {}
            qt = {}
            for t in "AB":
                kt[t] = bp.tile([128, N], BF, tag=f"kt_{t}", name=f"kt_{t}")
                vt[t] = bp.tile([128, N], BF, tag=f"vt_{t}", name=f"vt_{t}")
                qt[t] = bp.tile([128, R], BF, tag=f"qt_{t}", name=f"qt_{t}")

            def emit_k(t, j):
                ps = ps_sm.tile([128, 512], F32, tag="sm",
                                name=f"kps_{t}_{j}")
                nc.tensor.matmul(
                    ps[:], lhsT=W[f"wkT_{t}"],
                    rhs=x0comb[:, TOFF[t] + j * 512:TOFF[t] + (j + 1) * 512],
                    start=True, stop=True)
                nc.vector.tensor_copy(kt[t][:, j * 512:(j + 1) * 512], ps[:])

            def emit_v(t, vg):
                ps = ps_sm.tile([128, 512], F32, tag="sm",
                                name=f"vps_{t}_{vg}")
                for i in range(4):
                    nb = vg * 4 + i
                    nc.tensor.matmul(
                        ps[:, i * 128:(i + 1) * 128],
                        lhsT=x0comb[:, TOFF[t] + nb * 128:
                                     TOFF[t] + (nb + 1) * 128],
                        rhs=W[f"wvoT_{t}"], start=True, stop=True)
                nc.vector.tensor_copy(vt[t][:, vg * 512:(vg + 1) * 512],
                                      ps[:])

            def emit_q(t, j):
                ps = ps_sm.tile([128, 512], F32, tag="sm",
                                name=f"qps_{t}_{j}")
                nc.tensor.matmul(
                    ps[:], lhsT=W[f"wqT_{t}"],
                    rhs=x0q_s[t][:, j * 512:(j + 1) * 512],
                    start=True, stop=True)
                nc.scalar.activation(
                    qt[t][:, j * 512:(j + 1) * 512], ps[:],
                    mybir.ActivationFunctionType.Identity,
                    bias=C[f"bqs_{t}"], scale=float(SCALE))

            for t in "AB":
                emit_q(t, 0)
            # prefetch the src-half-0 count matrices of AB/BA during the
            # attention phase (AA0-3 + the half-1 tiles stream into freed
            # bufs). Triggered from the scalar queue after the Q copies so
            # the 8MB doesn't compete with x0's input DMA at kernel start.
            ct_sb = {}
            for g in ("AB", "BA"):
                for scg in range(4):
                    ct_t = ctp.tile([128, 8 * R], F8, tag="ct",
                                    name=f"ct_{g}_{scg}")
                    nc.sync.dma_start(
                        out=ct_t[:],
                        in_=ct[g][scg * 128:(scg + 1) * 128, :])
                    ct_sb[(g, scg)] = ct_t
            for j in (0, 1):
                for t in "AB":
                    emit_k(t, j)
            for vg in (0, 1):
                for t in "AB":
                    emit_v(t, vg)

            # ---------------- attention, A/B interleaved; h gathered per half
            hsb = None
            for qg in range(R // QG):
                q_sl = slice(qg * QG, (qg + 1) * QG)
                ut_ps = {}
                racc0 = {}
                racc1 = {}
                for t in "AB":
                    ut_ps[t] = ps_u.tile([128, QG], F32, tag="ut",
                                         name=f"utps_{t}_{qg}")
                    racc0[t] = bp.tile([128, 2 * QG], BF,
                                       tag=f"racc0_{t}_{qg}",
                                       name=f"racc0_{t}_{qg}")
                    racc1[t] = bp.tile([128, 2 * QG], BF,
                                       tag=f"racc1_{t}_{qg}",
                                       name=f"racc1_{t}_{qg}")
                for pr in range(KB // 2):
                    kb0 = 2 * pr
                    for t in "AB":
                        sc = ps_s.tile([128, 1024], F32, tag="sc",
                                       name=f"sc_{t}_{pr}")
                        nc.tensor.matmul(sc[:, :512],
                                         lhsT=kt[t][:, kb0 * 128:(kb0 + 1) * 128],
                                         rhs=qt[t][:, q_sl],
                                         start=True, stop=True)
                        nc.tensor.matmul(sc[:, 512:],
                                         lhsT=kt[t][:, (kb0 + 1) * 128:(kb0 + 2) * 128],
                                         rhs=qt[t][:, q_sl],
                                         start=True, stop=True)
                        pt = ptp.tile([128, 1024], BF, tag="pt",
                                      name=f"pt_{t}_{pr}")
                        nc.scalar.activation(pt[:], sc[:],
                                             mybir.ActivationFunctionType.Exp)
                        nc.tensor.matmul(ut_ps[t][:],
                                         lhsT=vt[t][:, kb0 * 128:(kb0 + 1) * 128],
                                         rhs=pt[:, :512],
                                         start=(pr == 0), stop=False)
                        nc.tensor.matmul(ut_ps[t][:],
                                         lhsT=vt[t][:, (kb0 + 1) * 128:(kb0 + 2) * 128],
                                         rhs=pt[:, 512:],
                                         start=False, stop=(pr == KB // 2 - 1))
                        if pr % 4 != 3:
                            if pr == 0:
                                nc.vector.tensor_copy(racc0[t][:], pt[:])
                            else:
                                nc.vector.tensor_add(racc0[t][:], racc0[t][:],
                                                     pt[:])
                        else:
                            if pr == 3:
                                nc.gpsimd.tensor_copy(racc1[t][:], pt[:])
                            else:
                                nc.gpsimd.tensor_tensor(racc1[t][:],
                                                        racc1[t][:],
                                                        pt[:],
                                                        op=mybir.AluOpType.add)
                    if qg == 0:
                        # dribble the remaining K/V tiles into the loop,
                        # staying ~4 prs ahead of their first consumer
                        idx = pr // 2 + 2
                        if idx <= 15:
                            et = "A" if pr % 2 == 0 else "B"
                            emit_k(et, idx)
                            emit_v(et, idx)
                        if pr == 5:
                            emit_q("A", 1)
                            emit_q("B", 1)

                # normalize + both orientations of h; row-major h -> h_loc.
                # Sub-major so a row range finishes for both types early,
                # letting the partial AllGathers launch mid-normalize.
                ut_sb = {}
                for t in "AB":
                    ut_sb[t] = bp.tile([128, QG], BF, tag=f"ut_sb_{t}",
                                       name=f"ut_sb_{t}_{qg}")
                    nc.vector.tensor_copy(ut_sb[t][:], ut_ps[t][:])
                if hsb is None:
                    hsb = bp.tile([128, 2 * N], BF, tag="big", name="hsb")

                def gather_piece(r0, r1, b0, b1):
                    # AllGather h_loc rows [r0, r1) of each core; land the
                    # result (permuted-space blocks [b0, b1)) into hsb,
                    # swizzled: partition = row-within-block, free = (blk, td)
                    nrow = r1 - r0
                    g0 = b0 * 128
                    nc.gpsimd.collective_compute(
                        "AllGather", mybir.AluOpType.bypass,
                        replica_groups=[list(range(NCORES))],
                        ins=[h_loc[r0:r1, :]],
                        outs=[h_all[g0:g0 + nrow * NCORES, :]])
                    for pc in range(b0 // 4, b1 // 4):
                        nc.sync.dma_start(
                            out=hsb[:, pc * 1024:(pc + 1) * 1024]
                            .rearrange("s (b d) -> s b d", d=256),
                            in_=h_all[pc * 512:(pc + 1) * 512, :]
                            .rearrange("(b s) d -> s b d", s=128))

                # batched: all row-sums, then transpose+scale+store (with
                # the partial gathers launched as their rows land), then the
                # feature-major re-transposes.
                rinv = {}
                for sub in range(QG // 128):
                    s_sl = slice(sub * 128, (sub + 1) * 128)
                    for t in "AB":
                        rp = ps_sm.tile([128, 512], F32, tag="sm", name="rp")
                        nc.tensor.matmul(rp[:, :1], lhsT=racc0[t][:, s_sl],
                                         rhs=ones_col[:], start=True,
                                         stop=False)
                        nc.tensor.matmul(rp[:, :1],
                                         lhsT=racc0[t][:, 512 + sub * 128:
                                                      512 + (sub + 1) * 128],
                                         rhs=ones_col[:], start=False,
                                         stop=False)
                        nc.tensor.matmul(rp[:, :1], lhsT=racc1[t][:, s_sl],
                                         rhs=ones_col[:], start=False,
                                         stop=False)
                        nc.tensor.matmul(rp[:, :1],
                                         lhsT=racc1[t][:, 512 + sub * 128:
                                                      512 + (sub + 1) * 128],
                                         rhs=ones_col[:], start=False,
                                         stop=True)
                        rv = bp.tile([128, 1], F32, tag="rinv", bufs=8,
                                     name=f"rinv_{t}_{sub}")
                        nc.vector.reciprocal(rv[:], rp[:, :1])
                        rinv[(t, sub)] = rv
                hns = {}
                for sub in range(QG // 128):
                    s_sl = slice(sub * 128, (sub + 1) * 128)
                    for t in "AB":
                        tyoff = 0 if t == "A" else 128
                        tp = ps_sm.tile([128, 512], BF, tag="sm", name="tp")
                        nc.tensor.transpose(tp[:, :128], ut_sb[t][:, s_sl],
                                            ident[:])
                        hn = bp.tile([128, 128], BF, tag=f"hn_{t}", bufs=4,
                                     name=f"hn_{t}_{sub}")
                        nc.vector.tensor_scalar_mul(hn[:], tp[:, :128],
                                                    rinv[(t, sub)][:, :])
                        nc.sync.dma_start(
                            out=h_loc[qg * QG + sub * 128:
                                      qg * QG + (sub + 1) * 128,
                                      tyoff:tyoff + 128],
                            in_=hn[:])
                        hns[(t, sub)] = hn

                # launch the gather as soon as the h_loc rows are written;
                # the ht re-transposes (only needed by the late wr matmuls)
                # follow it
                if qg == 0:
                    gather_piece(0, 512, 0, 32)
                else:
                    gather_piece(512, 1024, 32, 64)
                for sub in range(QG // 128):
                    for t in "AB":
                        tp2 = ps_sm.tile([128, 512], BF, tag="sm", name="tp2")
                        nc.tensor.transpose(tp2[:, :128], hns[(t, sub)][:],
                                            ident[:])
                        nc.vector.tensor_copy(
                            ht[t][:, qg * QG + sub * 128:
                                  qg * QG + (sub + 1) * 128],
                            tp2[:, :128])

            if dbg:
                for t in "AB":
                    nc.sync.dma_start(out=dbg_d[f"ht_{t}"][:], in_=ht[t][:])

            # ---------------- phase 2: dense count-matrix aggregation
            # agg_g^T[d, dst] = sum_blk h_blk^T @ C_g_blk ; then
            # out_t^T = sum_g wl_g @ agg_g^T + wr_t @ h_t^T  (corr on host)
            # Split per src-half: the half-0 matmuls only need AllGather#1,
            # so they run while AllGather#2 is still in flight.
            AGGSB_TAG = {"AB": "racc0_A_0", "BA": "racc0_B_0",
                         "AA": "racc1_A_0"}
            PIECES = ((0, 4), (4, 8))
            partial = {}
            aggsb = {}
            for pi in range(2 if stage >= 2 else 0):
                lo, hi = PIECES[pi]
                for g in G:
                    tyoff = 0 if SRC_T[g] == "A" else 128
                    agg = ps_s.tile([128, 1024], F32, tag="sc",
                                    name=f"agg_{g}_{pi}")
                    for scg in range(lo, hi):
                        if (g, scg) in ct_sb:
                            ct_t = ct_sb[(g, scg)]
                        else:
                            ct_t = ctp.tile([128, 8 * R], F8, tag="ct",
                                            name=f"ct_{g}_{scg}")
                            nc.sync.dma_start(
                                out=ct_t[:],
                                in_=ct[g][scg * 128:(scg + 1) * 128, :])
                        for sb in range(8):
                            blk = scg * 8 + sb
                            lt = hsb[:, blk * 256 + tyoff:
                                     blk * 256 + tyoff + 128]
                            for h in range(2):
                                nc.tensor.matmul(
                                    agg[:, h * 512:(h + 1) * 512],
                                    lhsT=lt,
                                    rhs=ct_t[:, sb * R + h * 512:
                                             sb * R + (h + 1) * 512],
                                    start=(scg == lo and sb == 0),
                                    stop=(scg == hi - 1 and sb == 7))
                    if pi == 0:
                        p = ptp.tile([128, 1024], BF, tag="pt",
                                     name=f"part_{g}")
                        nc.vector.tensor_copy(p[:], agg[:])
                        partial[g] = p
                    else:
                        asb = bp.tile([128, 1024], BF, tag=AGGSB_TAG[g],
                                      name=f"aggsb_{g}")
                        nc.vector.tensor_add(asb[:], partial[g][:], agg[:])
                        aggsb[g] = asb
                        # emit a target's output chain as soon as its last
                        # graph aggregate is ready (B after AB, A after AA)
                        t = {"AB": "B", "AA": "A"}.get(g)
                        if t is None:
                            continue
                        po = []
                        for h in range(2):
                            po_t = ps_u.tile([128, 512], F32, tag="ut",
                                             name=f"po_{t}_{h}")
                            po.append(po_t)
                        for h in range(2):
                            nc.tensor.matmul(
                                po[h][:], lhsT=W[f"wrT_{t}"],
                                rhs=ht[t][:, h * 512:(h + 1) * 512],
                                start=True, stop=False)
                        ggs = GRAPHS_OF[t]
                        for gg in ggs:
                            for h in range(2):
                                nc.tensor.matmul(
                                    po[h][:], lhsT=W[f"wlT_{gg}"],
                                    rhs=aggsb[gg][:, h * 512:(h + 1) * 512],
                                    start=False, stop=(gg == ggs[-1]))
                        for h in range(2):
                            osb = bp.tile([128, 512], BF, tag="osb",
                                          name=f"osb_{t}_{h}")
                            nc.vector.tensor_copy(osb[:], po[h][:])
                            nc.sync.dma_start(
                                out=out_d[t][:, h * 512:(h + 1) * 512],
                                in_=osb[:])

    nc.compile()
    return nc


# ---------------------------------------------------------------- host prep

def _row_perm():
    """node id -> gathered-h row under the half AllGather layout."""
    n = np.arange(N)
    c = n >> 10
    w = n & 1023
    return (w >> 9) * 4096 + c * 512 + (w & 511)


def _prep(inputs, dbg=False):
    ins = {k: np.asarray(v) for k, v in inputs.items()}

    def bf(x):
        return np.ascontiguousarray(np.asarray(x, np.float32)).astype(BF16)

    com = {}
    for t in "AB":
        iw = ins[f"inW_{t}"].astype(np.float32)
        ib = ins[f"inB_{t}"].astype(np.float32)
        ow = ins[f"outW_{t}"].astype(np.float32)
        ob = ins[f"outB_{t}"].astype(np.float32)
        com[f"wqT_{t}"] = iw[0:128].T
        com[f"wkT_{t}"] = iw[128:256].T
        com[f"wvoT_{t}"] = (ow @ iw[256:384]).T
        com[f"bqs_{t}"] = ib[0:128] * SCALE
        com[f"bk_{t}"] = ib[128:256]
        com[f"bout_eff_{t}"] = ow @ ib[256:384] + ob
    for g in G:
        com[f"wlT_{g}"] = ins[f"wl_{g}"].astype(np.float32).T
        com[f"c1_{g}"] = (ins[f"wl_{g}"].astype(np.float32)
                          @ com[f"bout_eff_{SRC_T[g]}"])
    com["wrT_B"] = ins["wr_AB"].astype(np.float32).T
    com["wrT_A"] = (ins["wr_BA"] + ins["wr_AA"]).astype(np.float32).T
    com["c0_B"] = (ins["bl_AB"].astype(np.float32)
                   + ins["wr_AB"].astype(np.float32) @ com["bout_eff_B"])
    com["c0_A"] = (ins["bl_BA"].astype(np.float32)
                   + ins["bl_AA"].astype(np.float32)
                   + (ins["wr_BA"] + ins["wr_AA"]).astype(np.float32)
                   @ com["bout_eff_A"])

    wblob = bf(np.concatenate([com[k] for k in WB_ORDER], axis=1))
    cblob = np.stack([com[k] for k in CB_ORDER], axis=1).astype(np.float32)

    x0T = {t: np.ascontiguousarray(
        ins[f"x_{t}"][:, 0, :].astype(np.float32).T).astype(BF16)
        for t in "AB"}

    perm = _row_perm()
    cts = {}
    degs = {}
    for g in G:
        src = np.asarray(ins[f"ei_{g}"][0], np.int64)
        dst = np.asarray(ins[f"ei_{g}"][1], np.int64)
        per_core = []
        dgs = []
        for c in range(NCORES):
            sel = (dst >> 10) == c
            s_c = perm[src[sel]]          # permuted gathered-h rows
            d_c = dst[sel] - c * R
            cmat = np.zeros((N, R), np.float32)
            np.add.at(cmat, (s_c, d_c), 1.0)
            swz = np.ascontiguousarray(
                cmat.reshape(8, 8, 128, R).transpose(0, 2, 1, 3)
                .reshape(1024, 8 * R))
            per_core.append(swz.astype(FP8))
            dgs.append(np.bincount(d_c, minlength=R).astype(np.float32))
        cts[g] = per_core
        degs[g] = dgs

    # host-side degree/bias correction, [R, 128] per (target, core)
    corr = {}
    for t in "AB":
        for c in range(NCORES):
            acc = np.broadcast_to(com[f"c0_{t}"], (R, 128)).copy()
            for g in GRAPHS_OF[t]:
                acc += np.outer(degs[g][c], com[f"c1_{g}"])
            corr[(t, c)] = acc.astype(np.float32)

    in_maps = []
    for c in range(NCORES):
        m = {"wblob": wblob, "cblob": cblob}
        for t in "AB":
            m[f"x0t_{t}"] = x0T[t]
            m[f"x0q_{t}"] = np.ascontiguousarray(x0T[t][:, c * R:(c + 1) * R])
        for g in G:
            m[f"ct_{g}"] = cts[g][c]
        in_maps.append(m)
    return in_maps, corr


def kernel(**inputs):
    in_maps, corr = _prep(inputs)
    if "prog" not in _PROG_CACHE:
        _PROG_CACHE["prog"] = build_program()
    nc = _PROG_CACHE["prog"]
    res = bass_utils.run_bass_kernel_spmd(
        nc, in_maps, core_ids=list(range(NCORES)))
    x_A = np.asarray(inputs["x_A"], np.float32)
    x_B = np.asarray(inputs["x_B"], np.float32)
    new_A = x_A.copy()
    new_B = x_B.copy()
    for c in range(NCORES):
        new_A[c * R:(c + 1) * R, 0, :] = (res.results[c]["out_A"].T
                                          + corr[("A", c)])
        new_B[c * R:(c + 1) * R, 0, :] = (res.results[c]["out_B"].T
                                          + corr[("B", c)])
    return new_A, new_B


# revision 34
# speedup vs baseline: 1.1122x; 1.0384x over previous
"""Trainium2 Bass kernel for nn_AttnDBGNNLayer (8-core SPMD).

kernel(**inputs) takes the FULL inputs (as produced by setup_inputs) and
returns the FULL output (new_A, new_B), distributing across 8 NeuronCores.

Design:
- q-rows of both attentions sharded 8-way (1024 rows/core); K/V computed
  replicated from a feature-major x0^T, with the K/V matmuls interleaved
  into the first attention q-group's loop (4-pr deadline lead) so the
  Scalar-engine EXP stream — the kernel's long pole — starts immediately;
  A and B q-groups interleaved so the TensorEngine always has independent
  work; single-pass unnormalized softmax (scores are tiny; no max
  subtraction; the K bias is dropped entirely — it adds a per-query
  constant to every score in a row, which row-softmax cancels exactly);
  out-projection folded into V (Wvo = Wout @ Wv); softmax row-sum
  accumulated on DVE+GpSimd (per-qg buffers, first-iteration copy instead
  of memset+add), finished with a ones-matmul; normalize batched
  (row-sums, then transpose+scale+store, then re-transposes) via
  PE-transpose + per-partition scale.
- normalized h rows (both types side by side, [R, 256]) are AllGathered in
  two row-halves so the first collective hides under the second attention
  q-group and the second under the half-0 aggregation; the gathered h
  lands in one SBUF tile (reusing the x0 buffer) swizzled so each 128-row
  source block is a matmul lhsT (8 parallel DMA chunks per half).
- message aggregation as dense count-matrix matmuls on RAW h:
  agg_g^T += h_blk^T @ C_g with C_g the per-core [8192 src, 1024 dst]
  edge-count matrix in fp8 (counts are small ints -> exact); lin_l (wl) is
  applied to the aggregate afterwards, so the three graphs share one
  gathered-h tile. C rows are host-permuted to match the AllGather row
  order and host-swizzled for contiguous streaming; the src-half-0 C of
  AB/BA is prefetched during the attention phase; aggregation runs in two
  src-half pieces (piece 0 needs only the first gather) with bf16 SBUF
  partials bridging the PSUM groups.
- degree/bias corrections (c0 + sum_g c1_g*deg_g) are rank-1/constant and
  are added on the host; lin_r accumulates into the same PSUM group as
  the wl matmuls; outputs staged bf16, transposed and corrected on host.
"""
import sys

if "/opt/trn_rl_repo" not in sys.path:
    sys.path.insert(0, "/opt/trn_rl_repo")

import numpy as np
import ml_dtypes

import concourse.bacc as bacc
import concourse.tile as tile
import concourse.mybir as mybir
from concourse import bass_utils

BF16 = ml_dtypes.bfloat16
FP8 = ml_dtypes.float8_e4m3

N = 8192
D = 128
NCORES = 8
R = N // NCORES       # 1024 rows per core
QG = 512              # q-group width
KB = N // 128         # 64 k-blocks
SCALE = 1.0 / np.sqrt(np.float32(D))

F32 = mybir.dt.float32
BF = mybir.dt.bfloat16
F8 = mybir.dt.float8e4

G = ("AB", "BA", "AA")
GI = {g: i for i, g in enumerate(G)}
SRC_T = {"AB": "A", "BA": "B", "AA": "A"}
GRAPHS_OF = {"A": ("BA", "AA"), "B": ("AB",)}

# bf16 weight blob layout: [128,128] slices
WB_ORDER = ["wqT_A", "wkT_A", "wvoT_A", "wqT_B", "wkT_B", "wvoT_B",
            "wlT_AB", "wlT_BA", "wlT_AA", "wrT_A", "wrT_B"]
# f32 col blob: [128, 4]
CB_ORDER = ["bqs_A", "bk_A", "bqs_B", "bk_B"]

_PROG_CACHE = {}


def build_program(dbg=False, stage=3):
    nc = bacc.Bacc("TRN2", target_bir_lowering=False, debug=False,
                   num_devices=NCORES)

    x0t = {t: nc.dram_tensor(f"x0t_{t}", [128, N], BF, kind="ExternalInput")
           for t in "AB"}
    x0q = {t: nc.dram_tensor(f"x0q_{t}", [128, R], BF, kind="ExternalInput")
           for t in "AB"}
    wblob = nc.dram_tensor("wblob", [128, 128 * len(WB_ORDER)], BF,
                           kind="ExternalInput")
    cblob = nc.dram_tensor("cblob", [128, len(CB_ORDER)], F32,
                           kind="ExternalInput")
    ct = {g: nc.dram_tensor(f"ct_{g}", [1024, 8 * R], F8,
                            kind="ExternalInput") for g in G}
    out_d = {t: nc.dram_tensor(f"out_{t}", [128, R], BF,
                               kind="ExternalOutput") for t in "AB"}
    dbg_d = {}
    if dbg:
        for t in "AB":
            dbg_d[f"ht_{t}"] = nc.dram_tensor(f"dbg_ht_{t}", [128, R], BF,
                                              kind="ExternalOutput")

    h_loc = nc.dram_tensor("h_loc", [R, 256], BF)
    h_all = nc.dram_tensor("h_all", [N, 256], BF, addr_space="Shared")

    with tile.TileContext(nc) as tc:
        with (
            tc.tile_pool(name="const", bufs=1) as cp,
            tc.tile_pool(name="big", bufs=1) as bp,
            tc.tile_pool(name="pt", bufs=4) as ptp,
            tc.tile_pool(name="ctp", bufs=8) as ctp,
            tc.tile_pool(name="ps_s", bufs=2, space="PSUM") as ps_s,
            tc.tile_pool(name="ps_u", bufs=2, space="PSUM") as ps_u,
            tc.tile_pool(name="ps_sm", bufs=2, space="PSUM") as ps_sm,
        ):
            # ---------------- inputs: small blobs first, then chunked x0
            wb = cp.tile([128, 128 * len(WB_ORDER)], BF, tag="wb")
            nc.sync.dma_start(out=wb[:], in_=wblob[:])
            W = {k: wb[:, i * 128:(i + 1) * 128]
                 for i, k in enumerate(WB_ORDER)}
            cb = cp.tile([128, len(CB_ORDER)], F32, tag="cb")
            nc.sync.dma_start(out=cb[:], in_=cblob[:])
            C = {k: cb[:, i:i + 1] for i, k in enumerate(CB_ORDER)}
            x0q_s = {}
            for t in "AB":
                x0q_s[t] = bp.tile([128, R], BF, tag=f"x0q_{t}",
                                   name=f"x0q_{t}")
                nc.sync.dma_start(out=x0q_s[t][:], in_=x0q[t][:])

            # x0comb holds x0^T of A (cols 0..8191) and B (cols 8192..);
            # the same 4MB buffer is later reused for the gathered h (hsb).
            x0comb = bp.tile([128, 2 * N], BF, tag="big", name="x0comb")
            for c0, c1 in ((0, 512), (512, 1024), (1024, 2048), (2048, 4096),
                           (4096, 6144), (6144, 8192)):
                for t, toff in (("A", 0), ("B", N)):
                    nc.sync.dma_start(
                        out=x0comb[:, toff + c0:toff + c1],
                        in_=x0t[t][:, c0:c1])

            ident = cp.tile([128, 128], BF, tag="ident")
            from concourse.masks import make_identity
            make_identity(nc, ident[:])
            ones_col = cp.tile([128, 1], BF, tag="ones_col")
            nc.vector.memset(ones_col[:], 1.0)

            ht = {t: bp.tile([128, R], BF, tag=f"ht_{t}", name=f"ht_{t}")
                  for t in "AB"}

            # ---------------- QKV emit helpers; the bulk is interleaved
            # into the qg0 attention loop so the Scalar EXP stream (the
            # kernel's long pole) starts as early as possible.
            TOFF = {"A": 0, "B": N}
            kt = {}
            vt = {}
            qt = {}
            for t in "AB":
                kt[t] = bp.tile([128, N], BF, tag=f"kt_{t}", name=f"kt_{t}")
                vt[t] = bp.tile([128, N], BF, tag=f"vt_{t}", name=f"vt_{t}")
                qt[t] = bp.tile([128, R], BF, tag=f"qt_{t}", name=f"qt_{t}")

            def emit_k(t, j):
                ps = ps_sm.tile([128, 512], F32, tag="sm",
                                name=f"kps_{t}_{j}")
                nc.tensor.matmul(
                    ps[:], lhsT=W[f"wkT_{t}"],
                    rhs=x0comb[:, TOFF[t] + j * 512:TOFF[t] + (j + 1) * 512],
                    start=True, stop=True)
                nc.vector.tensor_copy(kt[t][:, j * 512:(j + 1) * 512], ps[:])

            def emit_v(t, vg):
                ps = ps_sm.tile([128, 512], F32, tag="sm",
                                name=f"vps_{t}_{vg}")
                for i in range(4):
                    nb = vg * 4 + i
                    nc.tensor.matmul(
                        ps[:, i * 128:(i + 1) * 128],
                        lhsT=x0comb[:, TOFF[t] + nb * 128:
                                     TOFF[t] + (nb + 1) * 128],
                        rhs=W[f"wvoT_{t}"], start=True, stop=True)
                nc.vector.tensor_copy(vt[t][:, vg * 512:(vg + 1) * 512],
                                      ps[:])

            def emit_q(t, j):
                ps = ps_sm.tile([128, 512], F32, tag="sm",
                                name=f"qps_{t}_{j}")
                nc.tensor.matmul(
                    ps[:], lhsT=W[f"wqT_{t}"],
                    rhs=x0q_s[t][:, j * 512:(j + 1) * 512],
                    start=True, stop=True)
                nc.scalar.activation(
                    qt[t][:, j * 512:(j + 1) * 512], ps[:],
                    mybir.ActivationFunctionType.Identity,
                    bias=C[f"bqs_{t}"], scale=float(SCALE))

            for t in "AB":
                emit_q(t, 0)
            # prefetch the src-half-0 count matrices of AB/BA during the
            # attention phase (AA0-3 + the half-1 tiles stream into freed
            # bufs). Triggered from the scalar queue after the Q copies so
            # the 8MB doesn't compete with x0's input DMA at kernel start.
            ct_sb = {}
            for g in ("AB", "BA"):
                for scg in range(4):
                    ct_t = ctp.tile([128, 8 * R], F8, tag="ct",
                                    name=f"ct_{g}_{scg}")
                    nc.sync.dma_start(
                        out=ct_t[:],
                        in_=ct[g][scg * 128:(scg + 1) * 128, :])
                    ct_sb[(g, scg)] = ct_t
            for j in (0, 1):
                for t in "AB":
                    emit_k(t, j)
            for vg in (0, 1):
                for t in "AB":
                    emit_v(t, vg)

            # ---------------- attention, A/B interleaved; h gathered per half
            hsb = None
            for qg in range(R // QG):
                q_sl = slice(qg * QG, (qg + 1) * QG)
                ut_ps = {}
                racc0 = {}
                racc1 = {}
                for t in "AB":
                    ut_ps[t] = ps_u.tile([128, QG], F32, tag="ut",
                                         name=f"utps_{t}_{qg}")
                    racc0[t] = bp.tile([128, 2 * QG], BF,
                                       tag=f"racc0_{t}_{qg}",
                                       name=f"racc0_{t}_{qg}")
                    racc1[t] = bp.tile([128, 2 * QG], BF,
                                       tag=f"racc1_{t}_{qg}",
                                       name=f"racc1_{t}_{qg}")
                for pr in range(KB // 2):
                    kb0 = 2 * pr
                    for t in "AB":
                        sc = ps_s.tile([128, 1024], F32, tag="sc",
                                       name=f"sc_{t}_{pr}")
                        nc.tensor.matmul(sc[:, :512],
                                         lhsT=kt[t][:, kb0 * 128:(kb0 + 1) * 128],
                                         rhs=qt[t][:, q_sl],
                                         start=True, stop=True)
                        nc.tensor.matmul(sc[:, 512:],
                                         lhsT=kt[t][:, (kb0 + 1) * 128:(kb0 + 2) * 128],
                                         rhs=qt[t][:, q_sl],
                                         start=True, stop=True)
                        pt = ptp.tile([128, 1024], BF, tag="pt",
                                      name=f"pt_{t}_{pr}")
                        nc.scalar.activation(pt[:], sc[:],
                                             mybir.ActivationFunctionType.Exp)
                        nc.tensor.matmul(ut_ps[t][:],
                                         lhsT=vt[t][:, kb0 * 128:(kb0 + 1) * 128],
                                         rhs=pt[:, :512],
                                         start=(pr == 0), stop=False)
                        nc.tensor.matmul(ut_ps[t][:],
                                         lhsT=vt[t][:, (kb0 + 1) * 128:(kb0 + 2) * 128],
                                         rhs=pt[:, 512:],
                                         start=False, stop=(pr == KB // 2 - 1))
                        if pr % 4 != 3:
                            if pr == 0:
                                nc.vector.tensor_copy(racc0[t][:], pt[:])
                            else:
                                nc.vector.tensor_add(racc0[t][:], racc0[t][:],
                                                     pt[:])
                        else:
                            if pr == 3:
                                nc.gpsimd.tensor_copy(racc1[t][:], pt[:])
                            else:
                                nc.gpsimd.tensor_tensor(racc1[t][:],
                                                        racc1[t][:],
                                                        pt[:],
                                                        op=mybir.AluOpType.add)
                    if qg == 0:
                        # dribble the remaining K/V tiles into the loop,
                        # staying ~4 prs ahead of their first consumer
                        idx = pr // 2 + 2
                        if idx <= 15:
                            et = "A" if pr % 2 == 0 else "B"
                            emit_k(et, idx)
                            emit_v(et, idx)
                        if pr == 5:
                            emit_q("A", 1)
                            emit_q("B", 1)

                # normalize + both orientations of h; row-major h -> h_loc.
                # Sub-major so a row range finishes for both types early,
                # letting the partial AllGathers launch mid-normalize.
                ut_sb = {}
                for t in "AB":
                    ut_sb[t] = bp.tile([128, QG], BF, tag=f"ut_sb_{t}",
                                       name=f"ut_sb_{t}_{qg}")
                    nc.vector.tensor_copy(ut_sb[t][:], ut_ps[t][:])
                if hsb is None:
                    hsb = bp.tile([128, 2 * N], BF, tag="big", name="hsb")

                def gather_piece(r0, r1, b0, b1):
                    # AllGather h_loc rows [r0, r1) of each core; land the
                    # result (permuted-space blocks [b0, b1)) into hsb,
                    # swizzled: partition = row-within-block, free = (blk, td)
                    nrow = r1 - r0
                    g0 = b0 * 128
                    nc.gpsimd.collective_compute(
                        "AllGather", mybir.AluOpType.bypass,
                        replica_groups=[list(range(NCORES))],
                        ins=[h_loc[r0:r1, :]],
                        outs=[h_all[g0:g0 + nrow * NCORES, :]])
                    for pc in range(b0 // 4, b1 // 4):
                        nc.sync.dma_start(
                            out=hsb[:, pc * 1024:(pc + 1) * 1024]
                            .rearrange("s (b d) -> s b d", d=256),
                            in_=h_all[pc * 512:(pc + 1) * 512, :]
                            .rearrange("(b s) d -> s b d", s=128))

                # batched: all row-sums, then transpose+scale+store (with
                # the partial gathers launched as their rows land), then the
                # feature-major re-transposes.
                rinv = {}
                for sub in range(QG // 128):
                    s_sl = slice(sub * 128, (sub + 1) * 128)
                    for t in "AB":
                        rp = ps_sm.tile([128, 512], F32, tag="sm", name="rp")
                        nc.tensor.matmul(rp[:, :1], lhsT=racc0[t][:, s_sl],
                                         rhs=ones_col[:], start=True,
                                         stop=False)
                        nc.tensor.matmul(rp[:, :1],
                                         lhsT=racc0[t][:, 512 + sub * 128:
                                                      512 + (sub + 1) * 128],
                                         rhs=ones_col[:], start=False,
                                         stop=False)
                        nc.tensor.matmul(rp[:, :1], lhsT=racc1[t][:, s_sl],
                                         rhs=ones_col[:], start=False,
                                         stop=False)
                        nc.tensor.matmul(rp[:, :1],
                                         lhsT=racc1[t][:, 512 + sub * 128:
                                                      512 + (sub + 1) * 128],
                                         rhs=ones_col[:], start=False,
                                         stop=True)
                        rv = bp.tile([128, 1], F32, tag="rinv", bufs=8,
                                     name=f"rinv_{t}_{sub}")
                        nc.vector.reciprocal(rv[:], rp[:, :1])
                        rinv[(t, sub)] = rv
                hns = {}
                for sub in range(QG // 128):
                    s_sl = slice(sub * 128, (sub + 1) * 128)
                    for t in "AB":
                        tyoff = 0 if t == "A" else 128
                        tp = ps_sm.tile([128, 512], BF, tag="sm", name="tp")
                        nc.tensor.transpose(tp[:, :128], ut_sb[t][:, s_sl],
                                            ident[:])
                        hn = bp.tile([128, 128], BF, tag=f"hn_{t}", bufs=4,
                                     name=f"hn_{t}_{sub}")
                        nc.vector.tensor_scalar_mul(hn[:], tp[:, :128],
                                                    rinv[(t, sub)][:, :])
                        nc.sync.dma_start(
                            out=h_loc[qg * QG + sub * 128:
                                      qg * QG + (sub + 1) * 128,
                                      tyoff:tyoff + 128],
                            in_=hn[:])
                        hns[(t, sub)] = hn

                # launch the gather as soon as the h_loc rows are written;
                # the ht re-transposes (only needed by the late wr matmuls)
                # follow it
                if qg == 0:
                    gather_piece(0, 512, 0, 32)
                else:
                    gather_piece(512, 1024, 32, 64)
                for sub in range(QG // 128):
                    for t in "AB":
                        tp2 = ps_sm.tile([128, 512], BF, tag="sm", name="tp2")
                        nc.tensor.transpose(tp2[:, :128], hns[(t, sub)][:],
                                            ident[:])
                        nc.vector.tensor_copy(
                            ht[t][:, qg * QG + sub * 128:
                                  qg * QG + (sub + 1) * 128],
                            tp2[:, :128])

            if dbg:
                for t in "AB":
                    nc.sync.dma_start(out=dbg_d[f"ht_{t}"][:], in_=ht[t][:])

            # ---------------- phase 2: dense count-matrix aggregation
            # agg_g^T[d, dst] = sum_blk h_blk^T @ C_g_blk ; then
            # out_t^T = sum_g wl_g @ agg_g^T + wr_t @ h_t^T  (corr on host)
            # Split per src-half: the half-0 matmuls only need AllGather#1,
            # so they run while AllGather#2 is still in flight.
            AGGSB_TAG = {"AB": "racc0_A_0", "BA": "racc0_B_0",
                         "AA": "racc1_A_0"}
            PIECES = ((0, 4), (4, 8))
            partial = {}
            aggsb = {}
            for pi in range(2 if stage >= 2 else 0):
                lo, hi = PIECES[pi]
                for g in G:
                    tyoff = 0 if SRC_T[g] == "A" else 128
                    agg = ps_s.tile([128, 1024], F32, tag="sc",
                                    name=f"agg_{g}_{pi}")
                    for scg in range(lo, hi):
                        if (g, scg) in ct_sb:
                            ct_t = ct_sb[(g, scg)]
                        else:
                            ct_t = ctp.tile([128, 8 * R], F8, tag="ct",
                                            name=f"ct_{g}_{scg}")
                            nc.sync.dma_start(
                                out=ct_t[:],
                                in_=ct[g][scg * 128:(scg + 1) * 128, :])
                        for sb in range(8):
                            blk = scg * 8 + sb
                            lt = hsb[:, blk * 256 + tyoff:
                                     blk * 256 + tyoff + 128]
                            for h in range(2):
                                nc.tensor.matmul(
                                    agg[:, h * 512:(h + 1) * 512],
                                    lhsT=lt,
                                    rhs=ct_t[:, sb * R + h * 512:
                                             sb * R + (h + 1) * 512],
                                    start=(scg == lo and sb == 0),
                                    stop=(scg == hi - 1 and sb == 7))
                    if pi == 0:
                        p = ptp.tile([128, 1024], BF, tag="pt",
                                     name=f"part_{g}")
                        nc.vector.tensor_copy(p[:], agg[:])
                        partial[g] = p
                    else:
                        asb = bp.tile([128, 1024], BF, tag=AGGSB_TAG[g],
                                      name=f"aggsb_{g}")
                        nc.vector.tensor_add(asb[:], partial[g][:], agg[:])
                        aggsb[g] = asb
                        # emit a target's output chain as soon as its last
                        # graph aggregate is ready (B after AB, A after AA)
                        t = {"AB": "B", "AA": "A"}.get(g)
                        if t is None:
                            continue
                        po = []
                        for h in range(2):
                            po_t = ps_u.tile([128, 512], F32, tag="ut",
                                             name=f"po_{t}_{h}")
                            po.append(po_t)
                        for h in range(2):
                            nc.tensor.matmul(
                                po[h][:], lhsT=W[f"wrT_{t}"],
                                rhs=ht[t][:, h * 512:(h + 1) * 512],
                                start=True, stop=False)
                        ggs = GRAPHS_OF[t]
                        for gg in ggs:
                            for h in range(2):
                                nc.tensor.matmul(
                                    po[h][:], lhsT=W[f"wlT_{gg}"],
                                    rhs=aggsb[gg][:, h * 512:(h + 1) * 512],
                                    start=False, stop=(gg == ggs[-1]))
                        for h in range(2):
                            osb = bp.tile([128, 512], BF, tag="osb",
                                          name=f"osb_{t}_{h}")
                            nc.vector.tensor_copy(osb[:], po[h][:])
                            nc.sync.dma_start(
                                out=out_d[t][:, h * 512:(h + 1) * 512],
                                in_=osb[:])

    nc.compile()
    return nc


# ---------------------------------------------------------------- host prep

def _row_perm():
    """node id -> gathered-h row under the half AllGather layout."""
    n = np.arange(N)
    c = n >> 10
    w = n & 1023
    return (w >> 9) * 4096 + c * 512 + (w & 511)


def _prep(inputs, dbg=False):
    ins = {k: np.asarray(v) for k, v in inputs.items()}

    def bf(x):
        return np.ascontiguousarray(np.asarray(x, np.float32)).astype(BF16)

    com = {}
    for t in "AB":
        iw = ins[f"inW_{t}"].astype(np.float32)
        ib = ins[f"inB_{t}"].astype(np.float32)
        ow = ins[f"outW_{t}"].astype(np.float32)
        ob = ins[f"outB_{t}"].astype(np.float32)
        com[f"wqT_{t}"] = iw[0:128].T
        com[f"wkT_{t}"] = iw[128:256].T
        com[f"wvoT_{t}"] = (ow @ iw[256:384]).T
        com[f"bqs_{t}"] = ib[0:128] * SCALE
        com[f"bk_{t}"] = ib[128:256]
        com[f"bout_eff_{t}"] = ow @ ib[256:384] + ob
    for g in G:
        com[f"wlT_{g}"] = ins[f"wl_{g}"].astype(np.float32).T
        com[f"c1_{g}"] = (ins[f"wl_{g}"].astype(np.float32)
                          @ com[f"bout_eff_{SRC_T[g]}"])
    com["wrT_B"] = ins["wr_AB"].astype(np.float32).T
    com["wrT_A"] = (ins["wr_BA"] + ins["wr_AA"]).astype(np.float32).T
    com["c0_B"] = (ins["bl_AB"].astype(np.float32)
                   + ins["wr_AB"].astype(np.float32) @ com["bout_eff_B"])
    com["c0_A"] = (ins["bl_BA"].astype(np.float32)
                   + ins["bl_AA"].astype(np.float32)
                   + (ins["wr_BA"] + ins["wr_AA"]).astype(np.float32)
                   @ com["bout_eff_A"])

    wblob = bf(np.concatenate([com[k] for k in WB_ORDER], axis=1))
    cblob = np.stack([com[k] for k in CB_ORDER], axis=1).astype(np.float32)

    x0T = {t: np.ascontiguousarray(
        ins[f"x_{t}"][:, 0, :].astype(np.float32).T).astype(BF16)
        for t in "AB"}

    perm = _row_perm()
    cts = {}
    degs = {}
    for g in G:
        src = np.asarray(ins[f"ei_{g}"][0], np.int64)
        dst = np.asarray(ins[f"ei_{g}"][1], np.int64)
        per_core = []
        dgs = []
        for c in range(NCORES):
            sel = (dst >> 10) == c
            s_c = perm[src[sel]]          # permuted gathered-h rows
            d_c = dst[sel] - c * R
            cmat = np.zeros((N, R), np.float32)
            np.add.at(cmat, (s_c, d_c), 1.0)
            swz = np.ascontiguousarray(
                cmat.reshape(8, 8, 128, R).transpose(0, 2, 1, 3)
                .reshape(1024, 8 * R))
            per_core.append(swz.astype(FP8))
            dgs.append(np.bincount(d_c, minlength=R).astype(np.float32))
        cts[g] = per_core
        degs[g] = dgs

    # host-side degree/bias correction, [R, 128] per (target, core)
    corr = {}
    for t in "AB":
        for c in range(NCORES):
            acc = np.broadcast_to(com[f"c0_{t}"], (R, 128)).copy()
            for g in GRAPHS_OF[t]:
                acc += np.outer(degs[g][c], com[f"c1_{g}"])
            corr[(t, c)] = acc.astype(np.float32)

    in_maps = []
    for c in range(NCORES):
        m = {"wblob": wblob, "cblob": cblob}
        for t in "AB":
            m[f"x0t_{t}"] = x0T[t]
            m[f"x0q_{t}"] = np.ascontiguousarray(x0T[t][:, c * R:(c + 1) * R])
        for g in G:
            m[f"ct_{g}"] = cts[g][c]
        in_maps.append(m)
    return in_maps, corr


def kernel(**inputs):
    in_maps, corr = _prep(inputs)
    if "prog" not in _PROG_CACHE:
        _PROG_CACHE["prog"] = build_program()
    nc = _PROG_CACHE["prog"]
    res = bass_utils.run_bass_kernel_spmd(
        nc, in_maps, core_ids=list(range(NCORES)))
    x_A = np.asarray(inputs["x_A"], np.float32)
    x_B = np.asarray(inputs["x_B"], np.float32)
    new_A = x_A.copy()
    new_B = x_B.copy()
    for c in range(NCORES):
        new_A[c * R:(c + 1) * R, 0, :] = (res.results[c]["out_A"].T
                                          + corr[("A", c)])
        new_B[c * R:(c + 1) * R, 0, :] = (res.results[c]["out_B"].T
                                          + corr[("B", c)])
    return new_A, new_B
